# revision 11
# baseline (speedup 1.0000x reference)
"""Trainium2 Bass kernel for MipRayMarcher2 (NeuS-style ray compositing).

Contract: kernel(**inputs) takes FULL unsharded numpy inputs, shards the
ray axis across 8 NeuronCores (fully data-parallel), runs one SPMD Bass
program per core, and reassembles the full outputs.

Shapes (hardcoded): B=4, R=16384, S=48 samples/ray.
Outputs (matching the reference tuple):
  composite_rgb   [B,R,3]
  composite_depth [B,R,1]
  weights         [B,R,S-1,1]
  composite_normal[B,R,3]

Per-core layout: 128 partitions x G rays/partition per tile, samples on the
free dim. Host-side prep: colors/normals/real_normals are transposed to
channel-major [rays, 3, S] and downcast to bf16 (halves their DMA traffic;
the normals' bf16 error enters alpha only through iter_cos*delta/2, which
suppresses it ~30x; colors/realn only scale their own composited outputs).
sdfs/depths stay fp32 (the cdf chain needs them exact).

Engine split (from perfetto traces):
  GPSIMD  - contiguous 2-input mults/adds (nd, vc, vn, vd, dl, a2, e2)
  VectorE - q chain, alpha chain, fused segmented-scan transmittance,
            reductions, reciprocal_approx_fast (so ScalarE needs no Ln/Exp
            activation-table reloads, ~1.3us each)
  ScalarE - Sigmoid only
"""

import sys

for _p in ("/opt/trn_rl_repo", "/root/.axon_site/_ro/pypackages"):
    if _p not in sys.path:
        sys.path.insert(0, _p)

import ml_dtypes
import numpy as np

import concourse.bass as bass
import concourse.bacc as bacc
import concourse.tile as tile
from concourse import mybir
from concourse import bass_utils

# ---- problem constants --------------------------------------------------
B, R, S = 4, 16384, 48
SM = S - 1  # 47 mid samples
N_CORES = 8
RAYS = B * R                     # 65536
RAYS_PER_CORE = RAYS // N_CORES  # 8192
P = 128                          # partitions
G = 8                            # rays per partition per tile
TILE_RAYS = P * G                # 2048
N_TILES = RAYS_PER_CORE // TILE_RAYS  # 4

F32 = mybir.dt.float32
BF16 = mybir.dt.bfloat16
ALU = mybir.AluOpType
ACT = mybir.ActivationFunctionType
BF = ml_dtypes.bfloat16


def _build_program(k_half: float) -> bass.Bass:
    """Build the per-core Bass program. k_half = inv_std/2 (baked in)."""
    nc = bacc.Bacc("TRN2", target_bir_lowering=False, debug=False,
                   num_devices=N_CORES)

    # DRAM I/O (per-core shard, ray-flattened; colors/normals ch-major bf16)
    c_d = nc.dram_tensor("colors", [RAYS_PER_CORE, 3, S], BF16, kind="ExternalInput").ap()
    s_d = nc.dram_tensor("sdfs", [RAYS_PER_CORE, S], F32, kind="ExternalInput").ap()
    d_d = nc.dram_tensor("depths", [RAYS_PER_CORE, S], F32, kind="ExternalInput").ap()
    n_d = nc.dram_tensor("normals", [RAYS_PER_CORE, 3, S], F32, kind="ExternalInput").ap()
    rn_d = nc.dram_tensor("realn", [RAYS_PER_CORE, 3, S], BF16, kind="ExternalInput").ap()
    dir_d = nc.dram_tensor("dirs", [RAYS_PER_CORE, 3], F32, kind="ExternalInput").ap()

    rgb_d = nc.dram_tensor("rgb", [RAYS_PER_CORE, 3], F32, kind="ExternalOutput").ap()
    dep_d = nc.dram_tensor("dep", [RAYS_PER_CORE], F32, kind="ExternalOutput").ap()
    w_d = nc.dram_tensor("wout", [RAYS_PER_CORE, SM], F32, kind="ExternalOutput").ap()
    nrm_d = nc.dram_tensor("nrm", [RAYS_PER_CORE, 3], F32, kind="ExternalOutput").ap()

    # tiled views: ray = t*TILE_RAYS + p*G + g
    c_r = c_d.rearrange("(t p g) c s -> t p g c s", p=P, g=G)
    s_r = s_d.rearrange("(t p g) s -> t p g s", p=P, g=G)
    d_r = d_d.rearrange("(t p g) s -> t p g s", p=P, g=G)
    n_r = n_d.rearrange("(t p g) c s -> t p g c s", p=P, g=G)
    rn_r = rn_d.rearrange("(t p g) c s -> t p g c s", p=P, g=G)
    dir_r = dir_d.rearrange("(t p g) c -> t p g c", p=P, g=G)
    rgb_r = rgb_d.rearrange("(t p g) c -> t p g c", p=P, g=G)
    dep_r = dep_d.rearrange("(t p g) -> t p g", p=P, g=G)
    w_r = w_d.rearrange("(t p g) s -> t p g s", p=P, g=G)
    nrm_r = nrm_d.rearrange("(t p g) c -> t p g c", p=P, g=G)

    with tile.TileContext(nc) as tc:
        with (
            tc.tile_pool(name="consts", bufs=1) as consts,
            tc.tile_pool(name="ins", bufs=2) as ins,
            tc.tile_pool(name="tmp", bufs=2) as tmp,
            tc.tile_pool(name="outs", bufs=2) as outs,
        ):
            # segment-boundary mask for the fused transmittance scan:
            # 1.0 at s==0 of each group, 0 elsewhere
            bmask = consts.tile([P, G, S], F32)
            nc.vector.memset(bmask, 0.0)
            nc.vector.memset(bmask[:, :, 0:1], 1.0)

            for t in range(N_TILES):
                # ---- loads ----
                c_t = ins.tile([P, G, 3, S], BF16, tag="c")
                s_t = ins.tile([P, G, S], F32, tag="s")
                d_t = ins.tile([P, G, S], F32, tag="d")
                n_t = ins.tile([P, G, 3, S], F32, tag="n")
                rn_t = ins.tile([P, G, 3, S], BF16, tag="rn")
                dir_t = ins.tile([P, G, 3], F32, tag="dir")
                nc.sync.dma_start(out=c_t, in_=c_r[t])
                nc.sync.dma_start(out=s_t, in_=s_r[t])
                nc.sync.dma_start(out=d_t, in_=d_r[t])
                nc.sync.dma_start(out=n_t, in_=n_r[t])
                nc.sync.dma_start(out=rn_t, in_=rn_r[t])
                nc.sync.dma_start(out=dir_t, in_=dir_r[t])

                # ---- true_cos: q[s] = dot(dir, n[s]); all operands contiguous
                nd = tmp.tile([P, G, 3, S], F32, tag="nd")
                for ch in range(3):
                    dir_b = dir_t[:, :, ch].unsqueeze(2).to_broadcast([P, G, S])
                    nc.gpsimd.tensor_tensor(nd[:, :, ch, :], n_t[:, :, ch, :],
                                            dir_b, ALU.mult)
                q = tmp.tile([P, G, S], F32, tag="q")
                nc.vector.tensor_tensor(q, nd[:, :, 0, :], nd[:, :, 1, :], ALU.add)
                nc.vector.tensor_tensor(q, q, nd[:, :, 2, :], ALU.add)
                # tc2 = q[s]+q[s+1] = 2*cos_mid  (fp32 from here on)
                tc2 = tmp.tile([P, G, SM], F32, tag="tc2")
                nc.vector.tensor_tensor(tc2, q[:, :, 0:SM], q[:, :, 1:S], ALU.add)
                # m = max(-tc2, 0) = -2*iter_cos
                m = tmp.tile([P, G, SM], F32, tag="m")
                nc.vector.tensor_scalar(m, tc2, -1.0, 0.0, ALU.mult, ALU.max)

                # ---- deltas & 2*est sdfs: E+- = A2 +- (m/2)*delta
                dl = tmp.tile([P, G, SM], F32, tag="dl")
                nc.gpsimd.tensor_tensor(dl, d_t[:, :, 1:S], d_t[:, :, 0:SM], ALU.subtract)
                a2 = tmp.tile([P, G, SM], F32, tag="a2")
                nc.gpsimd.tensor_tensor(a2, s_t[:, :, 0:SM], s_t[:, :, 1:S], ALU.add)
                pp = tmp.tile([P, G, SM], F32, tag="pp")
                nc.vector.scalar_tensor_tensor(pp, m, 0.5, dl, ALU.mult, ALU.mult)
                e2 = tmp.tile([P, G, 2, SM], F32, tag="e2")
                nc.gpsimd.tensor_tensor(e2[:, :, 0, :], a2, pp, ALU.add)       # 2*est_prev
                nc.gpsimd.tensor_tensor(e2[:, :, 1, :], a2, pp, ALU.subtract)  # 2*est_next
                cdf = tmp.tile([P, G, 2, SM], F32, tag="cdf")
                nc.scalar.activation(cdf, e2, ACT.Sigmoid, scale=float(k_half))

                # ---- alpha = clip((prev-next+1e-5)/(prev+1e-5), 0, 1)
                num2 = tmp.tile([P, G, SM], F32, tag="num2")
                nc.vector.scalar_tensor_tensor(num2, cdf[:, :, 0, :], 1e-5,
                                               cdf[:, :, 1, :], ALU.add, ALU.subtract)
                den = tmp.tile([P, G, SM], F32, tag="den")
                nc.vector.tensor_scalar_add(den, cdf[:, :, 0, :], 1e-5)
                rden = tmp.tile([P, G, SM], F32, tag="rden")
                nc.vector.reciprocal_approx_fast(rden, den)
                alpha = tmp.tile([P, G, SM], F32, tag="alpha")
                nc.vector.tensor_tensor(alpha, num2, rden, ALU.mult)
                nc.vector.tensor_scalar(alpha, alpha, 1.0, 0.0, ALU.min, ALU.max)

                # ---- transmittance via ONE segmented scan over all groups:
                # z[g,0]=0, z[g,s]=om[g,s-1];  state = max(z*state, bmask)
                # bmask=1 at s==0 resets each segment exactly to 1.0f.
                z = tmp.tile([P, G, S], F32, tag="z")
                nc.vector.memset(z[:, :, 0:1], 0.0)
                nc.vector.tensor_scalar(z[:, :, 1:S], alpha, -1.0, 1.0000000001,
                                        ALU.mult, ALU.add)
                tx = tmp.tile([P, G, S], F32, tag="tx")
                nc.vector.tensor_tensor_scan(
                    tx.rearrange("p g s -> p (g s)"),
                    z.rearrange("p g s -> p (g s)"),
                    bmask.rearrange("p g s -> p (g s)"),
                    0.0, ALU.mult, ALU.max)

                # ---- w = alpha * tx;  wf padded with zeros at both ends
                wf = tmp.tile([P, G, SM + 2], F32, tag="wf")  # [0]=0, [1..47]=w, [48]=0
                nc.vector.memset(wf[:, :, 0:1], 0.0)
                nc.vector.memset(wf[:, :, SM + 1:SM + 2], 0.0)
                nc.vector.tensor_tensor(wf[:, :, 1:SM + 1], alpha, tx[:, :, 0:SM], ALU.mult)
                nc.sync.dma_start(out=w_r[t], in_=wf[:, :, 1:SM + 1])

                # ---- v[s] = w[s-1]+w[s]: composite = 0.5 * sum v*x; sum v = 2*wt
                v = tmp.tile([P, G, S], F32, tag="v")
                nc.vector.tensor_tensor(v, wf[:, :, 0:S], wf[:, :, 1:S + 1], ALU.add)

                vc = tmp.tile([P, G, 3, S], BF16, tag="vc")
                vn = tmp.tile([P, G, 3, S], BF16, tag="vn")
                vd = tmp.tile([P, G, S], F32, tag="vd")
                for ch in range(3):
                    nc.gpsimd.tensor_tensor(vc[:, :, ch, :], c_t[:, :, ch, :], v, ALU.mult)
                    nc.gpsimd.tensor_tensor(vn[:, :, ch, :], rn_t[:, :, ch, :], v, ALU.mult)
                nc.gpsimd.tensor_tensor(vd, d_t, v, ALU.mult)

                rgbs = tmp.tile([P, G, 3], F32, tag="rgbs")
                nrms = tmp.tile([P, G, 3], F32, tag="nrms")
                dsum = tmp.tile([P, G], F32, tag="dsum")
                wt2 = tmp.tile([P, G], F32, tag="wt2")
                nc.vector.tensor_reduce(rgbs, vc, mybir.AxisListType.X, ALU.add)
                nc.vector.tensor_reduce(nrms, vn, mybir.AxisListType.X, ALU.add)
                nc.vector.tensor_reduce(dsum, vd, mybir.AxisListType.X, ALU.add)
                # sum v = 2 * sum w  (exactly)
                nc.vector.tensor_reduce(wt2, v, mybir.AxisListType.X, ALU.add)

                # rwt2 = 1/(2*wt); wt2 in (~1e-5, 2], well inside approx range
                rwt2 = tmp.tile([P, G], F32, tag="rwt2")
                nc.vector.reciprocal_approx_fast(rwt2, wt2)

                rgb_o = outs.tile([P, G, 3], F32, tag="rgb_o")
                dep_o = outs.tile([P, G], F32, tag="dep_o")
                nrm_o = outs.tile([P, G, 3], F32, tag="nrm_o")
                nc.vector.tensor_scalar_mul(rgb_o, rgbs, 0.5)
                nc.vector.tensor_tensor(dep_o, dsum, rwt2, ALU.mult)
                rwt2_b = rwt2.unsqueeze(2).to_broadcast([P, G, 3])
                nc.vector.tensor_tensor(nrm_o, nrms, rwt2_b, ALU.mult)

                nc.sync.dma_start(out=rgb_r[t], in_=rgb_o)
                nc.sync.dma_start(out=dep_r[t], in_=dep_o)
                nc.sync.dma_start(out=nrm_r[t], in_=nrm_o)

    nc.compile()
    return nc


_PROGRAM_CACHE: dict[float, bass.Bass] = {}


def _get_program(k_half: float) -> bass.Bass:
    if k_half not in _PROGRAM_CACHE:
        _PROGRAM_CACHE[k_half] = _build_program(k_half)
    return _PROGRAM_CACHE[k_half]


def kernel(colors, sdfs, depths, normals, ray_directions, real_normals,
           inv_std_param, _trace=False):
    colors = np.asarray(colors, dtype=np.float32)
    sdfs = np.asarray(sdfs, dtype=np.float32)
    depths = np.asarray(depths, dtype=np.float32)
    normals = np.asarray(normals, dtype=np.float32)
    ray_directions = np.asarray(ray_directions, dtype=np.float32)
    real_normals = np.asarray(real_normals, dtype=np.float32)

    p = np.float32(np.asarray(inv_std_param).reshape(()))
    inv_std = np.clip(np.exp(np.float32(10.0) * p), np.float32(1e-6), np.float32(1e6))
    k_half = float(np.float32(inv_std) * np.float32(0.5))

    nc = _get_program(k_half)

    # host prep: flatten rays, channel-major + bf16 for color-like tensors
    cf = np.ascontiguousarray(
        colors.reshape(RAYS, S, 3).transpose(0, 2, 1)).astype(BF)
    nf = np.ascontiguousarray(
        normals.reshape(RAYS, S, 3).transpose(0, 2, 1))
    rnf = np.ascontiguousarray(
        real_normals.reshape(RAYS, S, 3).transpose(0, 2, 1)).astype(BF)
    dirf = ray_directions.reshape(RAYS, 3)
    sf = sdfs.reshape(RAYS, S)
    df = depths.reshape(RAYS, S)

    in_maps = []
    for k in range(N_CORES):
        lo, hi = k * RAYS_PER_CORE, (k + 1) * RAYS_PER_CORE
        in_maps.append({
            "colors": np.ascontiguousarray(cf[lo:hi]),
            "sdfs": np.ascontiguousarray(sf[lo:hi]),
            "depths": np.ascontiguousarray(df[lo:hi]),
            "normals": np.ascontiguousarray(nf[lo:hi]),
            "realn": np.ascontiguousarray(rnf[lo:hi]),
            "dirs": np.ascontiguousarray(dirf[lo:hi]),
        })

    res = bass_utils.run_bass_kernel_spmd(
        nc, in_maps, core_ids=list(range(N_CORES)), trace=_trace)

    rgb = np.concatenate([res.results[k]["rgb"] for k in range(N_CORES)], axis=0)
    dep = np.concatenate([res.results[k]["dep"] for k in range(N_CORES)], axis=0)
    w = np.concatenate([res.results[k]["wout"] for k in range(N_CORES)], axis=0)
    nrm = np.concatenate([res.results[k]["nrm"] for k in range(N_CORES)], axis=0)

    # faithful edge handling (no-ops for non-degenerate rays)
    dep = np.nan_to_num(dep, nan=np.inf)
    dep = np.clip(dep, depths.min(), depths.max())
    nrm = np.nan_to_num(nrm, nan=np.inf)
    nrm = np.clip(nrm, real_normals.min(), real_normals.max())

    out = (rgb.reshape(B, R, 3).astype(np.float32),
           dep.reshape(B, R, 1).astype(np.float32),
           w.reshape(B, R, SM, 1).astype(np.float32),
           nrm.reshape(B, R, 3).astype(np.float32))
    if _trace:
        return out, res
    return out


# revision 13
# speedup vs baseline: 1.0774x; 1.0774x over previous
"""Trainium2 Bass kernel for MipRayMarcher2 (NeuS-style ray compositing).

Contract: kernel(**inputs) takes FULL unsharded numpy inputs, shards the
ray axis across 8 NeuronCores (fully data-parallel), runs one SPMD Bass
program per core, and reassembles the full outputs.

Shapes (hardcoded): B=4, R=16384, S=48 samples/ray.
Outputs (matching the reference tuple):
  composite_rgb   [B,R,3]
  composite_depth [B,R,1]
  weights         [B,R,S-1,1]
  composite_normal[B,R,3]

Per-core layout: 128 partitions x G rays/partition per tile, samples on the
free dim. Host-side prep: colors/normals/real_normals are transposed to
channel-major [rays, 3, S] and downcast to bf16 (halves their DMA traffic;
the normals' bf16 error enters alpha only through iter_cos*delta/2, which
suppresses it ~30x; colors/realn only scale their own composited outputs).
sdfs/depths stay fp32 (the cdf chain needs them exact).

Engine split (from perfetto traces):
  GPSIMD  - contiguous 2-input mults/adds (nd, vc, vn, vd, dl, a2, e2)
  VectorE - q chain, alpha chain, fused segmented-scan transmittance,
            reductions, reciprocal_approx_fast (so ScalarE needs no Ln/Exp
            activation-table reloads, ~1.3us each)
  ScalarE - Sigmoid only
"""

import sys

for _p in ("/opt/trn_rl_repo", "/root/.axon_site/_ro/pypackages"):
    if _p not in sys.path:
        sys.path.insert(0, _p)

import ml_dtypes
import numpy as np

import concourse.bass as bass
import concourse.bacc as bacc
import concourse.tile as tile
from concourse import mybir
from concourse import bass_utils

# ---- problem constants --------------------------------------------------
B, R, S = 4, 16384, 48
SM = S - 1  # 47 mid samples
N_CORES = 8
RAYS = B * R                     # 65536
RAYS_PER_CORE = RAYS // N_CORES  # 8192
P = 128                          # partitions
G = 8                            # rays per partition per tile
TILE_RAYS = P * G                # 2048
N_TILES = RAYS_PER_CORE // TILE_RAYS  # 4

F32 = mybir.dt.float32
BF16 = mybir.dt.bfloat16
ALU = mybir.AluOpType
ACT = mybir.ActivationFunctionType
BF = ml_dtypes.bfloat16


def _build_program(k_half: float) -> bass.Bass:
    """Build the per-core Bass program. k_half = inv_std/2 (baked in)."""
    nc = bacc.Bacc("TRN2", target_bir_lowering=False, debug=False,
                   num_devices=N_CORES)

    # DRAM I/O (per-core shard, ray-flattened; colors/normals ch-major bf16)
    c_d = nc.dram_tensor("colors", [RAYS_PER_CORE, 3, S], BF16, kind="ExternalInput").ap()
    s_d = nc.dram_tensor("sdfs", [RAYS_PER_CORE, S], F32, kind="ExternalInput").ap()
    d_d = nc.dram_tensor("depths", [RAYS_PER_CORE, S], F32, kind="ExternalInput").ap()
    n_d = nc.dram_tensor("normals", [RAYS_PER_CORE, 3, S], F32, kind="ExternalInput").ap()
    rn_d = nc.dram_tensor("realn", [RAYS_PER_CORE, 3, S], BF16, kind="ExternalInput").ap()
    dir_d = nc.dram_tensor("dirs", [RAYS_PER_CORE, 3], F32, kind="ExternalInput").ap()

    rgb_d = nc.dram_tensor("rgb", [RAYS_PER_CORE, 3], F32, kind="ExternalOutput").ap()
    dep_d = nc.dram_tensor("dep", [RAYS_PER_CORE], F32, kind="ExternalOutput").ap()
    w_d = nc.dram_tensor("wout", [RAYS_PER_CORE, SM], F32, kind="ExternalOutput").ap()
    nrm_d = nc.dram_tensor("nrm", [RAYS_PER_CORE, 3], F32, kind="ExternalOutput").ap()

    # tiled views: ray = t*TILE_RAYS + p*G + g
    c_r = c_d.rearrange("(t p g) c s -> t p g c s", p=P, g=G)
    s_r = s_d.rearrange("(t p g) s -> t p g s", p=P, g=G)
    d_r = d_d.rearrange("(t p g) s -> t p g s", p=P, g=G)
    n_r = n_d.rearrange("(t p g) c s -> t p g c s", p=P, g=G)
    rn_r = rn_d.rearrange("(t p g) c s -> t p g c s", p=P, g=G)
    dir_r = dir_d.rearrange("(t p g) c -> t p g c", p=P, g=G)
    rgb_r = rgb_d.rearrange("(t p g) c -> t p g c", p=P, g=G)
    dep_r = dep_d.rearrange("(t p g) -> t p g", p=P, g=G)
    w_r = w_d.rearrange("(t p g) s -> t p g s", p=P, g=G)
    nrm_r = nrm_d.rearrange("(t p g) c -> t p g c", p=P, g=G)

    with tile.TileContext(nc) as tc:
        with (
            tc.tile_pool(name="consts", bufs=1) as consts,
            tc.tile_pool(name="ins", bufs=3) as ins,
            tc.tile_pool(name="tmp", bufs=3) as tmp,
            tc.tile_pool(name="outs", bufs=2) as outs,
        ):
            # segment-boundary mask for the fused transmittance scan:
            # 1.0 at s==0 of each group, 0 elsewhere
            bmask = consts.tile([P, G, S], F32)
            nc.vector.memset(bmask, 0.0)
            nc.vector.memset(bmask[:, :, 0:1], 1.0)

            for t in range(N_TILES):
                # ---- loads ----
                c_t = ins.tile([P, G, 3, S], BF16, tag="c")
                s_t = ins.tile([P, G, S], F32, tag="s")
                d_t = ins.tile([P, G, S], F32, tag="d")
                n_t = ins.tile([P, G, 3, S], F32, tag="n")
                rn_t = ins.tile([P, G, 3, S], BF16, tag="rn")
                dir_t = ins.tile([P, G, 3], F32, tag="dir")
                nc.sync.dma_start(out=c_t, in_=c_r[t])
                nc.sync.dma_start(out=s_t, in_=s_r[t])
                nc.sync.dma_start(out=d_t, in_=d_r[t])
                nc.sync.dma_start(out=n_t, in_=n_r[t])
                nc.sync.dma_start(out=rn_t, in_=rn_r[t])
                nc.sync.dma_start(out=dir_t, in_=dir_r[t])

                # ---- true_cos: q[s] = dot(dir, n[s]); all operands contiguous
                nd = tmp.tile([P, G, 3, S], F32, tag="nd")
                for ch in range(3):
                    dir_b = dir_t[:, :, ch].unsqueeze(2).to_broadcast([P, G, S])
                    nc.gpsimd.tensor_tensor(nd[:, :, ch, :], n_t[:, :, ch, :],
                                            dir_b, ALU.mult)
                q = tmp.tile([P, G, S], F32, tag="q")
                nc.gpsimd.tensor_tensor(q, nd[:, :, 0, :], nd[:, :, 1, :], ALU.add)
                nc.gpsimd.tensor_tensor(q, q, nd[:, :, 2, :], ALU.add)
                # tc2 = q[s]+q[s+1] = 2*cos_mid  (fp32 from here on)
                tc2 = tmp.tile([P, G, SM], F32, tag="tc2")
                nc.gpsimd.tensor_tensor(tc2, q[:, :, 0:SM], q[:, :, 1:S], ALU.add)
                # m = relu(-0.5*tc2) = -iter_cos  (relu is positive-homogeneous)
                m = tmp.tile([P, G, SM], F32, tag="m")
                nc.scalar.activation(m, tc2, ACT.Relu, scale=-0.5)

                # ---- deltas & 2*est sdfs: E+- = A2 +- (m/2)*delta
                dl = tmp.tile([P, G, SM], F32, tag="dl")
                nc.gpsimd.tensor_tensor(dl, d_t[:, :, 1:S], d_t[:, :, 0:SM], ALU.subtract)
                a2 = tmp.tile([P, G, SM], F32, tag="a2")
                nc.gpsimd.tensor_tensor(a2, s_t[:, :, 0:SM], s_t[:, :, 1:S], ALU.add)
                pp = tmp.tile([P, G, SM], F32, tag="pp")
                nc.gpsimd.tensor_tensor(pp, m, dl, ALU.mult)
                e2 = tmp.tile([P, G, 2, SM], F32, tag="e2")
                nc.gpsimd.tensor_tensor(e2[:, :, 0, :], a2, pp, ALU.add)       # 2*est_prev
                nc.gpsimd.tensor_tensor(e2[:, :, 1, :], a2, pp, ALU.subtract)  # 2*est_next
                cdf = tmp.tile([P, G, 2, SM], F32, tag="cdf")
                nc.scalar.activation(cdf, e2, ACT.Sigmoid, scale=float(k_half))

                # ---- alpha = clip((prev-next+1e-5)/(prev+1e-5), 0, 1)
                num2 = tmp.tile([P, G, SM], F32, tag="num2")
                nc.vector.scalar_tensor_tensor(num2, cdf[:, :, 0, :], 1e-5,
                                               cdf[:, :, 1, :], ALU.add, ALU.subtract)
                den = tmp.tile([P, G, SM], F32, tag="den")
                nc.vector.tensor_scalar_add(den, cdf[:, :, 0, :], 1e-5)
                rden = tmp.tile([P, G, SM], F32, tag="rden")
                nc.vector.reciprocal_approx_fast(rden, den)
                alpha = tmp.tile([P, G, SM], F32, tag="alpha")
                nc.vector.tensor_tensor(alpha, num2, rden, ALU.mult)
                nc.vector.tensor_scalar(alpha, alpha, 1.0, 0.0, ALU.min, ALU.max)

                # ---- transmittance via ONE segmented scan over all groups:
                # z[g,0]=0, z[g,s]=om[g,s-1];  state = max(z*state, bmask)
                # bmask=1 at s==0 resets each segment exactly to 1.0f.
                z = tmp.tile([P, G, S], F32, tag="z")
                nc.vector.memset(z[:, :, 0:1], 0.0)
                nc.vector.tensor_scalar(z[:, :, 1:S], alpha, -1.0, 1.0000000001,
                                        ALU.mult, ALU.add)
                tx = tmp.tile([P, G, S], F32, tag="tx")
                nc.vector.tensor_tensor_scan(
                    tx.rearrange("p g s -> p (g s)"),
                    z.rearrange("p g s -> p (g s)"),
                    bmask.rearrange("p g s -> p (g s)"),
                    0.0, ALU.mult, ALU.max)

                # ---- w = alpha * tx;  wf padded with zeros at both ends
                wf = tmp.tile([P, G, SM + 2], F32, tag="wf")  # [0]=0, [1..47]=w, [48]=0
                nc.vector.memset(wf[:, :, 0:1], 0.0)
                nc.vector.memset(wf[:, :, SM + 1:SM + 2], 0.0)
                nc.vector.tensor_tensor(wf[:, :, 1:SM + 1], alpha, tx[:, :, 0:SM], ALU.mult)
                nc.sync.dma_start(out=w_r[t], in_=wf[:, :, 1:SM + 1])

                # ---- v[s] = w[s-1]+w[s]: composite = 0.5 * sum v*x; sum v = 2*wt
                v = tmp.tile([P, G, S], F32, tag="v")
                nc.vector.tensor_tensor(v, wf[:, :, 0:S], wf[:, :, 1:S + 1], ALU.add)

                v_b = tmp.tile([P, G, S], BF16, tag="v_b")
                nc.vector.tensor_copy(v_b, v)
                vc = tmp.tile([P, G, 3, S], BF16, tag="vc")
                vn = tmp.tile([P, G, 3, S], BF16, tag="vn")
                vd = tmp.tile([P, G, S], F32, tag="vd")
                for ch in range(3):
                    nc.vector.tensor_tensor(vc[:, :, ch, :], c_t[:, :, ch, :], v_b, ALU.mult)
                    nc.vector.tensor_tensor(vn[:, :, ch, :], rn_t[:, :, ch, :], v_b, ALU.mult)
                nc.vector.tensor_tensor(vd, d_t, v, ALU.mult)

                rgbs = tmp.tile([P, G, 3], F32, tag="rgbs")
                nrms = tmp.tile([P, G, 3], F32, tag="nrms")
                dsum = tmp.tile([P, G], F32, tag="dsum")
                wt2 = tmp.tile([P, G], F32, tag="wt2")
                nc.vector.tensor_reduce(rgbs, vc, mybir.AxisListType.X, ALU.add)
                nc.vector.tensor_reduce(nrms, vn, mybir.AxisListType.X, ALU.add)
                nc.vector.tensor_reduce(dsum, vd, mybir.AxisListType.X, ALU.add)
                # sum v = 2 * sum w  (exactly)
                nc.vector.tensor_reduce(wt2, v, mybir.AxisListType.X, ALU.add)

                # rwt2 = 1/(2*wt); wt2 in (~1e-5, 2], well inside approx range
                rwt2 = tmp.tile([P, G], F32, tag="rwt2")
                nc.vector.reciprocal_approx_fast(rwt2, wt2)

                rgb_o = outs.tile([P, G, 3], F32, tag="rgb_o")
                dep_o = outs.tile([P, G], F32, tag="dep_o")
                nrm_o = outs.tile([P, G, 3], F32, tag="nrm_o")
                nc.vector.tensor_scalar_mul(rgb_o, rgbs, 0.5)
                nc.vector.tensor_tensor(dep_o, dsum, rwt2, ALU.mult)
                rwt2_b = rwt2.unsqueeze(2).to_broadcast([P, G, 3])
                nc.vector.tensor_tensor(nrm_o, nrms, rwt2_b, ALU.mult)

                nc.sync.dma_start(out=rgb_r[t], in_=rgb_o)
                nc.sync.dma_start(out=dep_r[t], in_=dep_o)
                nc.sync.dma_start(out=nrm_r[t], in_=nrm_o)

    nc.compile()
    return nc


_PROGRAM_CACHE: dict[float, bass.Bass] = {}


def _get_program(k_half: float) -> bass.Bass:
    if k_half not in _PROGRAM_CACHE:
        _PROGRAM_CACHE[k_half] = _build_program(k_half)
    return _PROGRAM_CACHE[k_half]


def kernel(colors, sdfs, depths, normals, ray_directions, real_normals,
           inv_std_param, _trace=False):
    colors = np.asarray(colors, dtype=np.float32)
    sdfs = np.asarray(sdfs, dtype=np.float32)
    depths = np.asarray(depths, dtype=np.float32)
    normals = np.asarray(normals, dtype=np.float32)
    ray_directions = np.asarray(ray_directions, dtype=np.float32)
    real_normals = np.asarray(real_normals, dtype=np.float32)

    p = np.float32(np.asarray(inv_std_param).reshape(()))
    inv_std = np.clip(np.exp(np.float32(10.0) * p), np.float32(1e-6), np.float32(1e6))
    k_half = float(np.float32(inv_std) * np.float32(0.5))

    nc = _get_program(k_half)

    # host prep: flatten rays, channel-major + bf16 for color-like tensors
    cf = np.ascontiguousarray(
        colors.reshape(RAYS, S, 3).transpose(0, 2, 1)).astype(BF)
    nf = np.ascontiguousarray(
        normals.reshape(RAYS, S, 3).transpose(0, 2, 1))
    rnf = np.ascontiguousarray(
        real_normals.reshape(RAYS, S, 3).transpose(0, 2, 1)).astype(BF)
    dirf = ray_directions.reshape(RAYS, 3)
    sf = sdfs.reshape(RAYS, S)
    df = depths.reshape(RAYS, S)

    in_maps = []
    for k in range(N_CORES):
        lo, hi = k * RAYS_PER_CORE, (k + 1) * RAYS_PER_CORE
        in_maps.append({
            "colors": np.ascontiguousarray(cf[lo:hi]),
            "sdfs": np.ascontiguousarray(sf[lo:hi]),
            "depths": np.ascontiguousarray(df[lo:hi]),
            "normals": np.ascontiguousarray(nf[lo:hi]),
            "realn": np.ascontiguousarray(rnf[lo:hi]),
            "dirs": np.ascontiguousarray(dirf[lo:hi]),
        })

    res = bass_utils.run_bass_kernel_spmd(
        nc, in_maps, core_ids=list(range(N_CORES)), trace=_trace)

    rgb = np.concatenate([res.results[k]["rgb"] for k in range(N_CORES)], axis=0)
    dep = np.concatenate([res.results[k]["dep"] for k in range(N_CORES)], axis=0)
    w = np.concatenate([res.results[k]["wout"] for k in range(N_CORES)], axis=0)
    nrm = np.concatenate([res.results[k]["nrm"] for k in range(N_CORES)], axis=0)

    # faithful edge handling (no-ops for non-degenerate rays)
    dep = np.nan_to_num(dep, nan=np.inf)
    dep = np.clip(dep, depths.min(), depths.max())
    nrm = np.nan_to_num(nrm, nan=np.inf)
    nrm = np.clip(nrm, real_normals.min(), real_normals.max())

    out = (rgb.reshape(B, R, 3).astype(np.float32),
           dep.reshape(B, R, 1).astype(np.float32),
           w.reshape(B, R, SM, 1).astype(np.float32),
           nrm.reshape(B, R, 3).astype(np.float32))
    if _trace:
        return out, res
    return out
